# revision 1
# baseline (speedup 1.0000x reference)
"""Trainium2 Bass kernel for GQA attention block (B=2, S=2048, HS=2048, H=16, HKV=4, D=128).

Strategy (8 NeuronCores, SPMD):
  - Head-parallel: core c computes q-heads {2c, 2c+1} and kv-head c//2 for BOTH batches.
  - Fused QKV projection: one 512-wide rhs stream [q0|q1|k|v] per contraction tile.
  - Per-head RMS norm + RoPE in [tok, d] layout (norm weights and 1/sqrt(D) folded
    into host-precomputed cos/sin tables), then PE-transpose to [d, tok].
  - Causal flash attention in transposed layout: S^T = K_rope @ Q_rope^T ([kv, q]),
    exp on ScalarE (no max subtraction needed: |scores| <= sqrt(D)), binary causal
    mask by multiply on diagonal blocks, O^T = V^T @ P^T accumulated in PSUM,
    softmax denominators via ones-vector matmul, normalization via K=1 broadcast
    matmul + vector multiply.
  - One 8-rank AllToAll redistributes head-shards -> (batch, seq-strip) shards.
  - Output projection per strip; host concatenates the 8 strips.
"""

import sys

sys.path.insert(0, "/opt/trn_rl_repo")

import numpy as np
import ml_dtypes

BF16 = ml_dtypes.bfloat16

B, H, HKV, D = 2, 16, 4, 128
EPS = 1e-6
P = 128
N_CORES = 8


def build(S=2048, HS=2048, probe="full"):
    """Build + compile the SPMD graph. Returns the Bacc module.

    probe: "full" | "s12" (qkv+norm+rope only) | "s3" (skip collective)
    """
    import concourse.bacc as bacc
    import concourse.tile as tile
    import concourse.mybir as mybir

    dt = mybir.dt
    f32 = dt.float32
    bf16 = dt.bfloat16
    AF = mybir.ActivationFunctionType
    ALU = mybir.AluOpType

    T = S // P          # tok tiles per batch
    M = 2 * T           # tok tiles total (2 batches)
    KT = HS // P        # contraction tiles for qkv projection
    KO = (H * D) // P   # contraction tiles for o projection (16)
    CW = S // 4         # q-chunk width == strip width
    CB = CW // P        # kv blocks per chunk step
    OCH = HS // 512     # output column chunks
    NQ = 2              # q heads per core

    nc = bacc.Bacc("TRN2", target_bir_lowering=False, debug=False,
                   enable_asserts=True, num_devices=N_CORES)

    xT = nc.dram_tensor("xT", [M, P, HS], bf16, kind="ExternalInput")
    wqkvT = nc.dram_tensor("wqkvT", [P, KT * 512], bf16, kind="ExternalInput")
    woT = nc.dram_tensor("woT", [P, KO * HS], bf16, kind="ExternalInput")
    cosq_d = nc.dram_tensor("cosq", [P, T * D], bf16, kind="ExternalInput")
    sinq_d = nc.dram_tensor("sinq", [P, T * D], bf16, kind="ExternalInput")
    cosk_d = nc.dram_tensor("cosk", [P, T * D], bf16, kind="ExternalInput")
    sink_d = nc.dram_tensor("sink", [P, T * D], bf16, kind="ExternalInput")
    masks_d = nc.dram_tensor("masks", [P, CB, CW], bf16, kind="ExternalInput")
    onesq_d = nc.dram_tensor("onesq", [P, P], bf16, kind="ExternalInput")
    ident_d = nc.dram_tensor("ident", [P, P], bf16, kind="ExternalInput")
    out_d = nc.dram_tensor("out", [CW, HS], f32, kind="ExternalOutput")

    with tile.TileContext(nc) as tc:
        with tc.tile_pool(name="const", bufs=1) as cpool, \
             tc.tile_pool(name="weights", bufs=1) as wpool, \
             tc.tile_pool(name="qkv", bufs=1) as qkvpool, \
             tc.tile_pool(name="xin", bufs=4) as xin, \
             tc.tile_pool(name="dram", bufs=1, space="DRAM") as dpool:

            xms = {}

            def load_xm(m):
                t_ = xin.tile([P, KT, P], bf16, tag="xm", name=f"xm{m}")
                nc.sync.dma_start(t_[:], xT.ap()[m].rearrange("p (k t) -> p k t", k=KT))
                xms[m] = t_

            for _m in range(3):
                load_xm(_m)

            wqkv_sb = wpool.tile([P, KT, 512], bf16, name="wqkv_sb")
            nc.sync.dma_start(wqkv_sb[:],
                              wqkvT.ap().rearrange("p (k f) -> p k f", k=KT))

            cosq_sb = cpool.tile([P, T, D], bf16, name="cosq_sb")
            sinq_sb = cpool.tile([P, T, D], bf16, name="sinq_sb")
            cosk_sb = cpool.tile([P, T, D], bf16, name="cosk_sb")
            sink_sb = cpool.tile([P, T, D], bf16, name="sink_sb")
            nc.sync.dma_start(cosq_sb[:], cosq_d.ap().rearrange("p (t d) -> p t d", t=T))
            nc.sync.dma_start(sinq_sb[:], sinq_d.ap().rearrange("p (t d) -> p t d", t=T))
            nc.sync.dma_start(cosk_sb[:], cosk_d.ap().rearrange("p (t d) -> p t d", t=T))
            nc.sync.dma_start(sink_sb[:], sink_d.ap().rearrange("p (t d) -> p t d", t=T))
            masks_sb = cpool.tile([P, CB, CW], bf16, name="masks_sb")
            nc.sync.dma_start(masks_sb[:], masks_d.ap())
            onesq_sb = cpool.tile([P, P], bf16, name="onesq_sb")
            nc.sync.dma_start(onesq_sb[:], onesq_d.ap())
            ident_sb = cpool.tile([P, P], bf16, name="ident_sb")
            nc.sync.dma_start(ident_sb[:], ident_d.ap())
            eps_sb = cpool.tile([P, 1], f32, name="eps_sb")
            nc.gpsimd.memset(eps_sb[:], EPS)

            qT_sb = qkvpool.tile([P, NQ, 2 * S], bf16, name="qT_sb")
            kT_sb = qkvpool.tile([P, 2 * S], bf16, name="kT_sb")
            v_sb = qkvpool.tile([P, M, D], bf16, name="v_sb")

            a2a_in = [dpool.tile([1024, CW], bf16, name=f"a2a_in{h}")
                      for h in range(NQ)]
            a2a_out = [dpool.tile([1024, CW], bf16, name=f"a2a_out{h}")
                       for h in range(NQ)]

            # ---------------- stage 1+2: QKV projection, RMS norm, RoPE, transpose
            with tc.tile_pool(name="s12", bufs=2) as s12, \
                 tc.tile_pool(name="ps12", bufs=2, space="PSUM") as ps12:
                for m in range(M):
                    b, mm = m // T, m % T
                    if m + 3 < M:
                        load_xm(m + 3)
                    xm = xms.pop(m)
                    ps_qkv = ps12.tile([P, 512], f32, tag="qkv")
                    for k in range(KT):
                        nc.tensor.matmul(ps_qkv, xm[:, k, :], wqkv_sb[:, k, :],
                                         start=(k == 0), stop=(k == KT - 1))
                    # V: plain copy to [tok, d] layout
                    nc.scalar.copy(v_sb[:, m, :], ps_qkv[:, 384:512])
                    col = S * b + P * mm
                    if probe == "s1":
                        nc.scalar.copy(qT_sb[:, 0, col:col + P], ps_qkv[:, 0:P])
                        nc.scalar.copy(qT_sb[:, 1, col:col + P], ps_qkv[:, P:2 * P])
                        nc.scalar.copy(kT_sb[:, col:col + P], ps_qkv[:, 2 * P:3 * P])
                        continue
                    # squared-mean for q0|q1|k read straight from PSUM
                    sq = s12.tile([P, P], bf16, tag="sq")
                    ssum = s12.tile([P, 3], f32, tag="ssum")
                    for idx in range(3):
                        nc.scalar.activation(sq, ps_qkv[:, idx * P:(idx + 1) * P],
                                             AF.Square,
                                             accum_out=ssum[:, idx:idx + 1])
                    rms = s12.tile([P, 3], f32, tag="rms")
                    nc.scalar.activation(rms, ssum, AF.Sqrt,
                                         bias=eps_sb[:], scale=1.0 / D)
                    rinv = s12.tile([P, 3], f32, tag="rinv")
                    nc.vector.reciprocal_approx_fast(rinv, rms)
                    # normalized q (both heads) and k, read from PSUM with ACT scale
                    qs = s12.tile([P, 2 * P], bf16, tag="qs")
                    nc.scalar.mul(qs[:, 0:P], ps_qkv[:, 0:P], rinv[:, 0:1])
                    nc.scalar.mul(qs[:, P:2 * P], ps_qkv[:, P:2 * P], rinv[:, 1:2])
                    ks = s12.tile([P, P], bf16, tag="ks")
                    nc.scalar.mul(ks, ps_qkv[:, 2 * P:3 * P], rinv[:, 2:3])
                    # RoPE, q heads batched via duplicated tables
                    ro = s12.tile([P, 2 * P], bf16, tag="ro")
                    ro_v = ro.rearrange("p (h d) -> p h d", h=2)
                    qs_hv = qs.rearrange("p (h d) -> p h d", h=2)
                    cos_b = cosq_sb[:, mm, None, :].to_broadcast((P, 2, D))
                    nc.vector.tensor_tensor(ro_v, qs_hv, cos_b, ALU.mult)
                    rh = s12.tile([P, 2 * P], bf16, tag="rh")
                    rh_v = rh.rearrange("p (h x d) -> p h x d", h=2, x=2)
                    qs_v = qs.rearrange("p (h x d) -> p h x d", h=2, x=2)
                    sinq_mm = sinq_sb[:, mm, :].rearrange("p (x d) -> p x d", x=2)
                    sinA = sinq_mm[:, None, 0, :].to_broadcast((P, 2, 64))
                    sinB = sinq_mm[:, None, 1, :].to_broadcast((P, 2, 64))
                    nc.vector.tensor_tensor(rh_v[:, :, 0, :], qs_v[:, :, 1, :],
                                            sinA, ALU.mult)
                    nc.vector.tensor_tensor(rh_v[:, :, 1, :], qs_v[:, :, 0, :],
                                            sinB, ALU.mult)
                    nc.vector.tensor_tensor(ro, ro, rh, ALU.add)
                    # RoPE for k
                    rok = s12.tile([P, P], bf16, tag="rok")
                    nc.vector.tensor_tensor(rok, ks, cosk_sb[:, mm, :], ALU.mult)
                    rhk = s12.tile([P, P], bf16, tag="rhk")
                    nc.vector.tensor_tensor(rhk[:, 0:64], ks[:, 64:128],
                                            sink_sb[:, mm, 0:64], ALU.mult)
                    nc.vector.tensor_tensor(rhk[:, 64:128], ks[:, 0:64],
                                            sink_sb[:, mm, 64:128], ALU.mult)
                    nc.vector.tensor_tensor(rok, rok, rhk, ALU.add)
                    # transpose to [d, tok]
                    for idx in range(3):
                        srct = [ro[:, 0:P], ro[:, P:2 * P], rok][idx]
                        dst = qT_sb[:, idx, col:col + P] if idx < 2 \
                            else kT_sb[:, col:col + P]
                        tp = ps12.tile([P, P], bf16, tag="tp")
                        nc.tensor.transpose(tp, srct, ident_sb)
                        nc.scalar.copy(dst, tp)

            early = probe in ("s12", "s1")
            if early:
                with tc.tile_pool(name="pr", bufs=2) as pr:
                    for (src, r0) in ((qT_sb[:, 0, 0:HS], 0), (kT_sb[:, 0:HS], P)):
                        ptile = pr.tile([P, HS], f32, tag="ptile")
                        nc.scalar.copy(ptile, src)
                        nc.sync.dma_start(out_d.ap()[r0:r0 + P, :], ptile)

            # prefetch o-projection weights during attention
            wo_sb, _wo_free = tc.tile([P, KO, HS], bf16, name="wo_sb")
            wo_src = woT.ap().rearrange("p (k f) -> p k f", k=KO)
            for k4 in range(0, KO, 4):
                nc.sync.dma_start(wo_sb[:, k4:k4 + 4, :], wo_src[:, k4:k4 + 4, :])

            # ---------------- stage 3: causal attention, head-major so each
            # head's AllToAll overlaps the next head's compute
            with tc.tile_pool(name="s3", bufs=4) as s3, \
                 tc.tile_pool(name="s3b", bufs=2) as s3b, \
                 tc.tile_pool(name="ps3", bufs=2, space="PSUM") as ps3:
                for h in range(NQ if not early else 0):
                    for b in range(2):
                        for c in (3, 2, 1, 0):
                            qv = qT_sb[:, h, S * b + CW * c: S * b + CW * (c + 1)]
                            nb = (c + 1) * CB
                            o_ps = ps3.tile([P, CW], f32, tag="o")
                            sum_ps = ps3.tile([P, CW], f32, tag="sum")
                            for kb in range(nb):
                                s_ps = ps3.tile([P, CW], f32, tag="s", bufs=3)
                                nc.tensor.matmul(
                                    s_ps, kT_sb[:, S * b + P * kb: S * b + P * (kb + 1)],
                                    qv, start=True, stop=True)
                                pT = s3.tile([P, CW], bf16, tag="pT")
                                nc.scalar.activation(pT, s_ps, AF.Exp)
                                if kb >= c * CB:
                                    nc.vector.tensor_tensor(
                                        pT, pT, masks_sb[:, kb - c * CB, :], ALU.mult)
                                nc.tensor.matmul(o_ps, v_sb[:, T * b + kb, :], pT,
                                                 start=(kb == 0), stop=(kb == nb - 1))
                                nc.tensor.matmul(sum_ps, onesq_sb, pT,
                                                 start=(kb == 0), stop=(kb == nb - 1))
                            rec = s3b.tile([P, CW], f32, tag="rec")
                            nc.vector.reciprocal_approx_fast(rec, sum_ps)
                            o_sb = s3b.tile([P, CW], bf16, tag="o_sb")
                            nc.vector.tensor_tensor(o_sb, o_ps, rec, ALU.mult)
                            r0 = P * (4 * b + c)
                            nc.sync.dma_start(a2a_in[h][r0:r0 + P, :], o_sb)
                    if probe == "full":
                        nc.gpsimd.collective_compute(
                            "AllToAll", mybir.AluOpType.bypass,
                            ins=[a2a_in[h][:].opt()], outs=[a2a_out[h][:].opt()],
                            replica_groups=[list(range(N_CORES))],
                        )
            if probe == "s3":
                a2a_out = a2a_in

            # ---------------- stage 4: output projection for this core's strip
            with tc.tile_pool(name="s4", bufs=1) as s4, \
                 tc.tile_pool(name="s4o", bufs=2) as s4o, \
                 tc.tile_pool(name="ps4", bufs=2, space="PSUM") as ps4:
                attn_sb = []
                for h in range(NQ if not early else 0):
                    a_sb = s4.tile([P, KO // NQ, CW], bf16, name=f"attn_sb{h}")
                    nc.sync.dma_start(
                        a_sb[:], a2a_out[h][:].rearrange("(k p) t -> p k t", p=P))
                    attn_sb.append(a_sb)
                accs = {}
                for t in range(0 if early else CW // P):
                    for oc in range(OCH):
                        ps_o = ps4.tile([P, 512], f32, tag="oproj")
                        for k8 in range(KO // NQ):
                            nc.tensor.matmul(
                                ps_o, attn_sb[0][:, k8, P * t:P * (t + 1)],
                                wo_sb[:, NQ * k8, 512 * oc:512 * (oc + 1)],
                                start=(k8 == 0), stop=(k8 == KO // NQ - 1))
                        acc = s4.tile([P, 512], f32, tag="acc", bufs=16)
                        nc.scalar.copy(acc, ps_o)
                        accs[(t, oc)] = acc
                for t in range(0 if early else CW // P):
                    for oc in range(OCH):
                        ps_o = ps4.tile([P, 512], f32, tag="oproj")
                        for k8 in range(KO // NQ):
                            nc.tensor.matmul(
                                ps_o, attn_sb[1][:, k8, P * t:P * (t + 1)],
                                wo_sb[:, NQ * k8 + 1, 512 * oc:512 * (oc + 1)],
                                start=(k8 == 0), stop=(k8 == KO // NQ - 1))
                        osb = s4o.tile([P, 512], f32, tag="osb")
                        nc.vector.tensor_tensor(osb, ps_o, accs[(t, oc)], ALU.add)
                        nc.sync.dma_start(
                            out_d.ap()[P * t:P * (t + 1), 512 * oc:512 * (oc + 1)], osb)
            _wo_free()

    nc.compile()
    return nc


def shard_inputs(inputs, S=2048, HS=2048):
    """Full problem inputs -> list of 8 per-core in_maps (host-side prep)."""
    x = np.asarray(inputs["x"], np.float32)
    cos = np.asarray(inputs["cos"], np.float32)
    sin = np.asarray(inputs["sin"], np.float32)
    wq = np.asarray(inputs["wq"], np.float32)
    wk = np.asarray(inputs["wk"], np.float32)
    wv = np.asarray(inputs["wv"], np.float32)
    wo = np.asarray(inputs["wo"], np.float32)
    qw = np.asarray(inputs["q_norm_w"], np.float32)
    kw = np.asarray(inputs["k_norm_w"], np.float32)

    T = S // P
    M = 2 * T
    CW = S // 4
    CB = CW // P

    KT = HS // P
    xT_t = np.ascontiguousarray(
        x.reshape(M, P, KT, P).transpose(0, 3, 2, 1).reshape(M, P, HS)).astype(BF16)

    sgn = np.concatenate([-np.ones(64, np.float32), np.ones(64, np.float32)])
    scale = 1.0 / np.sqrt(D)

    def tile_p(a):
        # [(n*P), inner] row-major -> [P, n*inner] partition-major
        n = a.shape[0] // P
        return np.ascontiguousarray(
            a.reshape(n, P, a.shape[1]).transpose(1, 0, 2).reshape(P, -1))

    def fold(w, s):
        w_rot = np.concatenate([w[64:], w[:64]])
        c = tile_p((cos * w[None, :] * s).astype(np.float32)).astype(BF16)
        sn = tile_p((sin * (w_rot * sgn)[None, :] * s).astype(np.float32)).astype(BF16)
        return c, sn

    cosq, sinq = fold(qw, scale)
    cosk, sink = fold(kw, 1.0)

    r = np.arange(P)[:, None]
    t = np.arange(CW)[None, :]
    masks = np.stack([(r <= t - P * j) for j in range(CB)], axis=1).astype(BF16)

    onesq = np.ones((P, P), BF16)
    ident = np.eye(P, dtype=np.float32).astype(BF16)
    woT = tile_p(np.ascontiguousarray(wo.T)).astype(BF16)

    in_maps = []
    for c in range(N_CORES):
        kvh = c // 2
        wq_c = wq[2 * c * D:(2 * c + 2) * D]       # [256, HS]
        wk_c = wk[kvh * D:(kvh + 1) * D]           # [128, HS]
        wv_c = wv[kvh * D:(kvh + 1) * D]           # [128, HS]
        wqkv = np.concatenate([wq_c, wk_c, wv_c], axis=0)  # [512, HS]
        wqkvT = tile_p(np.ascontiguousarray(wqkv.T)).astype(BF16)  # [P, KT*512]
        in_maps.append({
            "xT": xT_t, "wqkvT": wqkvT, "woT": woT,
            "cosq": cosq, "sinq": sinq, "cosk": cosk, "sink": sink,
            "masks": masks, "onesq": onesq, "ident": ident,
        })
    return in_maps


def assemble(outs, S=2048, HS=2048):
    """Per-core strip outputs -> full [B, S, HS] output."""
    CW = S // 4
    full = np.empty((B, S, HS), np.float32)
    for c in range(N_CORES):
        full[c // 4, (c % 4) * CW:(c % 4 + 1) * CW, :] = outs[c]
    return full


_CACHE = {}


def _get_compiled(S=2048, HS=2048, probe="full"):
    key = (S, HS, probe)
    if key not in _CACHE:
        _CACHE[key] = build(S, HS, probe)
    return _CACHE[key]


def _ensure_ntff_hook():
    """The image's antenv lacks axon_hooks; synthesize it so trace=True works."""
    import types
    try:
        from antenv.axon_hooks import get_axon_ntff_profile_hook  # noqa: F401
        return
    except ImportError:
        pass
    import antenv
    from trn_agent_boot.trn_boot import _ntff_profile_via_ctypes
    mod = types.ModuleType("antenv.axon_hooks")
    mod._hook = _ntff_profile_via_ctypes("/opt/axon/libaxon_pjrt.so")
    mod.set_axon_ntff_profile_hook = lambda h: setattr(mod, "_hook", h)
    mod.get_axon_ntff_profile_hook = lambda: mod._hook
    sys.modules["antenv.axon_hooks"] = mod
    antenv.axon_hooks = mod


def run(inputs, S=2048, HS=2048, trace=False, tmpdir=None, probe="full"):
    import concourse.bass_utils as bu
    if trace:
        _ensure_ntff_hook()
        bu.upload_artifacts = lambda d: ""  # no artifact bucket in this container
    nc = _get_compiled(S, HS, probe)
    in_maps = shard_inputs(inputs, S, HS)
    res = bu.run_bass_kernel_spmd(nc, in_maps, core_ids=list(range(N_CORES)),
                                  trace=trace, tmpdir=tmpdir)
    out = assemble([r["out"] for r in res.results], S, HS)
    return out, res.exec_time_ns


def kernel(**inputs):
    out, _ = run(inputs)
    return out

